# revision 6
# baseline (speedup 1.0000x reference)
"""Trainium2 Bass kernel for nn_AttentionBlock (B=4, S=2048, D=1024, single head).

Sharding: 8 cores = 4 batches x 2 query-halves. Each core computes, for its
batch b and query half h:
  - V [2048, D] for the full key sequence (spilled to DRAM, streamed back)
  - QT [D, 1024] for its 1024 queries (query columns placed first in xT)
  - KT in 512-key chunks, fused with scoresT = KT.T @ QT and exp
    (no max subtraction; scores are O(1) so exp cannot overflow)
  - rowsum[q] via matmul with a ones vector; r = 1/rowsum
  - OT [D, 512] = V.T @ expT per query half, accumulated across all of PSUM
  - y = (OT.T @ WprojT) * r + bias

All matmuls fp32 (PE streams 1 col/cycle regardless of dtype). Keys are
permuted per-core (own half first) - attention is permutation invariant.

SBUF tags are manually aliased across phases (pool memory is the static sum
over tags): w0..7 carry wv -> wq -> ktc -> wp; xt0..7 carry xT -> V reload;
qt0..7 carry QT -> OT; vstage carries V staging -> bias/y staging.
PSUM uses explicit bank tags b0..b7.
"""

import numpy as np
from contextlib import ExitStack

D = 1024
S = 2048
SQ = 1024  # queries per core
P = 128
SCALE = float(1.0 / np.sqrt(np.float32(D)).astype(np.float32))

_CACHED = {}


def _build_nc():
    import concourse.bass as bass
    import concourse.tile as tile
    from concourse import bacc, mybir

    DT = mybir.dt.float32
    FP = mybir.dt.float32
    Exp = mybir.ActivationFunctionType.Exp
    MUL = mybir.AluOpType.mult
    ADD = mybir.AluOpType.add

    nc = bacc.Bacc("TRN2", target_bir_lowering=False)
    xt_d = nc.declare_dram_parameter("xt", [D, S], DT, isOutput=False)
    wq_d = nc.declare_dram_parameter("wqt", [D, D], DT, isOutput=False)
    wk_d = nc.declare_dram_parameter("wkt", [D, D], DT, isOutput=False)
    wv_d = nc.declare_dram_parameter("wvt", [D, D], DT, isOutput=False)
    wp_d = nc.declare_dram_parameter("wpt", [D, D], DT, isOutput=False)
    bias_d = nc.declare_dram_parameter("bias", [P, D], DT, isOutput=False)
    ones_d = nc.declare_dram_parameter("ones", [P, 1], DT, isOutput=False)
    y_d = nc.declare_dram_parameter("y", [SQ, D], DT, isOutput=True)

    ND = D // P     # 8 tiles along D
    NS = S // P     # 16 tiles along S

    with tile.TileContext(nc) as tc:
        with ExitStack() as ctx:
            pool = ctx.enter_context(tc.tile_pool(name="main", bufs=1))
            psum = ctx.enter_context(tc.tile_pool(name="psum", bufs=1, space="PSUM"))
            dram = ctx.enter_context(tc.tile_pool(name="dram", bufs=1, space="DRAM"))

            def ptile(shape, name, tag, bufs=1):
                return pool.tile(shape, DT, name=name, tag=tag, bufs=bufs)

            def bank(i, name, shape=(P, 512)):
                return psum.tile(list(shape), FP, name=name, tag=f"b{i}", bufs=1)

            # ---- resident inputs ----
            xt = []
            for d in range(ND):
                t = ptile([P, S], f"xt{d}", f"xt{d}")
                nc.sync.dma_start(t[:], xt_d[d * P:(d + 1) * P, :])
                xt.append(t)
            ones_sb = ptile([P, 1], "ones", "ones")
            nc.sync.dma_start(ones_sb[:], ones_d[:, :])

            def load_wblk(src):
                blks = []
                for d in range(ND):
                    t = ptile([P, D], f"w{d}", f"w{d}")
                    nc.sync.dma_start(t[:], src[d * P:(d + 1) * P, :])
                    blks.append(t)
                return blks

            v_spill = dram.tile([S, D], DT, name="vspill", tag="vspill")

            # ---- phase 1: V[sk][128, D] = sum_d xt[d][:, sk].T @ wv[d]; spill ----
            wv = load_wblk(wv_d)
            for sk in range(NS):
                pv0 = bank(2 * (sk % 2), f"pv0_{sk}")
                pv1 = bank(2 * (sk % 2) + 1, f"pv1_{sk}")
                for d in range(ND):
                    lt = xt[d][:, sk * P:(sk + 1) * P]
                    nc.tensor.matmul(pv0, lt, wv[d][:, 0:512],
                                     start=(d == 0), stop=(d == ND - 1))
                    nc.tensor.matmul(pv1, lt, wv[d][:, 512:1024],
                                     start=(d == 0), stop=(d == ND - 1))
                vb = ptile([P, D], f"vstage{sk}", "vstage", bufs=2)
                nc.vector.tensor_copy(vb[:, 0:512], pv0)
                nc.vector.tensor_copy(vb[:, 512:1024], pv1)
                nc.sync.dma_start(v_spill[sk * P:(sk + 1) * P, :], vb[:])

            # ---- phase 2: QT[e][128, SQ] = sum_d wq[d][:, e].T @ xt[d][:, :SQ] ----
            wq = load_wblk(wq_d)
            qt = []
            for e in range(ND):
                qt.append(ptile([P, SQ], f"qt{e}", f"qt{e}"))
            for e in range(ND):
                ps0 = bank(2 * (e % 2), f"pq0_{e}")
                ps1 = bank(2 * (e % 2) + 1, f"pq1_{e}")
                for d in range(ND):
                    lt = wq[d][:, e * P:(e + 1) * P]
                    nc.tensor.matmul(ps0, lt, xt[d][:, 0:512],
                                     start=(d == 0), stop=(d == ND - 1))
                    nc.tensor.matmul(ps1, lt, xt[d][:, 512:1024],
                                     start=(d == 0), stop=(d == ND - 1))
                nc.vector.tensor_copy(qt[e][:, 0:512], ps0)
                nc.vector.tensor_copy(qt[e][:, 512:1024], ps1)

            # ---- phase 3: KT chunks (wk streamed as 128x128 tiles) fused with
            #      scoresT + exp; expT[sk][128, SQ] resident ----
            expT = []
            for sk in range(NS):
                expT.append(ptile([P, SQ], f"expT{sk}", f"expT{sk}"))
            for c in range(4):  # 512-key chunk
                ktc = []
                for e in range(ND):
                    pk = bank(4 + (e % 2), f"pk_{c}_{e}")
                    for d in range(ND):
                        wkt = ptile([P, P], f"wks_{c}_{e}_{d}", "wks", bufs=6)
                        nc.sync.dma_start(
                            wkt[:], wk_d[d * P:(d + 1) * P, e * P:(e + 1) * P])
                        nc.tensor.matmul(pk, wkt[:], xt[d][:, c * 512:(c + 1) * 512],
                                         start=(d == 0), stop=(d == ND - 1))
                    kt_sb = ptile([P, 512], f"ktc_{c}_{e}", f"w{e}")
                    nc.vector.tensor_copy(kt_sb[:], pk)
                    ktc.append(kt_sb)
                for t in range(4):
                    sk = c * 4 + t
                    ps0 = bank(t % 2, f"psc0_{sk}")
                    ps1 = bank(2 + t % 2, f"psc1_{sk}")
                    for e in range(ND):
                        lt = ktc[e][:, t * P:(t + 1) * P]
                        nc.tensor.matmul(ps0, lt, qt[e][:, 0:512],
                                         start=(e == 0), stop=(e == ND - 1))
                        nc.tensor.matmul(ps1, lt, qt[e][:, 512:1024],
                                         start=(e == 0), stop=(e == ND - 1))
                    nc.scalar.activation(expT[sk][:, 0:512], ps0, Exp, scale=SCALE)
                    nc.scalar.activation(expT[sk][:, 512:1024], ps1, Exp, scale=SCALE)

            # ---- phase 4: rowsum -> r [128, 8] (col j: 1/rowsum, query tile j) ----
            psr = bank(6, "psr", shape=(P, ND))
            for j in range(ND):
                for sk in range(NS):
                    nc.tensor.matmul(psr[:, j:j + 1],
                                     expT[sk][:, j * P:(j + 1) * P], ones_sb[:],
                                     start=(sk == 0), stop=(sk == NS - 1))
            r_sb = ptile([P, ND], "rsb", "rsb")
            nc.vector.reciprocal(r_sb[:], psr[:])

            # ---- phases 5+6 per query half: OT in PSUM (all 8 banks), proj ----
            wp = load_wblk(wp_d)
            bias_sb = ptile([P, D], "bias", "vstage", bufs=2)
            nc.sync.dma_start(bias_sb[:], bias_d[:, :])
            for h in range(2):
                otp = [bank(e, f"ot_{h}_{e}") for e in range(ND)]
                for sk in range(NS):
                    vb = ptile([P, D], f"vload_{h}_{sk}", f"xt{sk % 3}")
                    nc.sync.dma_start(vb[:], v_spill[sk * P:(sk + 1) * P, :])
                    for e in range(ND):
                        nc.tensor.matmul(otp[e], vb[:, e * P:(e + 1) * P],
                                         expT[sk][:, h * 512:(h + 1) * 512],
                                         start=(sk == 0), stop=(sk == NS - 1))
                ot = []
                for e in range(ND):
                    t = ptile([P, 512], f"ot_sb_{h}_{e}", f"qt{e}")
                    nc.vector.tensor_copy(t[:], otp[e])
                    ot.append(t)
                for sqt in range(4):
                    g = h * 4 + sqt
                    ps0 = bank(2 * (sqt % 2), f"py0_{g}")
                    ps1 = bank(2 * (sqt % 2) + 1, f"py1_{g}")
                    for e in range(ND):
                        lt = ot[e][:, sqt * P:(sqt + 1) * P]
                        nc.tensor.matmul(ps0, lt, wp[e][:, 0:512],
                                         start=(e == 0), stop=(e == ND - 1))
                        nc.tensor.matmul(ps1, lt, wp[e][:, 512:1024],
                                         start=(e == 0), stop=(e == ND - 1))
                    ysb = ptile([P, D], f"ysb_{g}", "vstage", bufs=2)
                    rcol = r_sb[:, g:g + 1]
                    nc.vector.scalar_tensor_tensor(ysb[:, 0:512], ps0, rcol,
                                                   bias_sb[:, 0:512], MUL, ADD)
                    nc.vector.scalar_tensor_tensor(ysb[:, 512:1024], ps1, rcol,
                                                   bias_sb[:, 512:1024], MUL, ADD)
                    nc.sync.dma_start(y_d[g * P:(g + 1) * P, :], ysb[:])

    nc.compile()
    return nc


def _get_nc():
    if "nc" not in _CACHED:
        _CACHED["nc"] = _build_nc()
    return _CACHED["nc"]


def make_in_maps(x, w_qkv, w_proj, b_proj):
    wqT = np.ascontiguousarray(w_qkv[0:D].T)
    wkT = np.ascontiguousarray(w_qkv[D:2 * D].T)
    wvT = np.ascontiguousarray(w_qkv[2 * D:3 * D].T)
    wpT = np.ascontiguousarray(w_proj.T)
    bias = np.ascontiguousarray(np.broadcast_to(b_proj[None, :], (P, D)))
    ones = np.ones((P, 1), dtype=np.float32)
    in_maps = []
    for c in range(8):
        b, h = c // 2, c % 2
        own = x[b, h * SQ:(h + 1) * SQ]       # [1024, D] our queries
        other = x[b, (1 - h) * SQ:(2 - h) * SQ]
        xt = np.ascontiguousarray(np.concatenate([own.T, other.T], axis=1))
        in_maps.append({
            "xt": xt, "wqt": wqT, "wkt": wkT, "wvt": wvT, "wpt": wpT,
            "bias": bias, "ones": ones,
        })
    return in_maps


def gather_out(results):
    out = np.empty((4, S, D), dtype=np.float32)
    for c in range(8):
        b, h = c // 2, c % 2
        out[b, h * SQ:(h + 1) * SQ] = results[c]["y"]
    return out


def kernel(x, w_qkv, w_proj, b_proj):
    from concourse import bass_utils
    nc = _get_nc()
    in_maps = make_in_maps(np.asarray(x, dtype=np.float32),
                           np.asarray(w_qkv, dtype=np.float32),
                           np.asarray(w_proj, dtype=np.float32),
                           np.asarray(b_proj, dtype=np.float32))
    res = bass_utils.run_bass_kernel_spmd(nc, in_maps, list(range(8))).results
    return gather_out(res)


# revision 11
# speedup vs baseline: 3.3573x; 3.3573x over previous
"""Trainium2 Bass kernel for nn_AttentionBlock (B=4, S=2048, D=1024, single head).

Sharding: 8 cores = 4 batches x 2 query-halves. Each core computes, for its
batch b and query half h:
  - V [2048, D] for the full key sequence (spilled to DRAM, streamed back)
  - QT [D, 1024] for its 1024 queries (query columns placed first in xT)
  - KT in 1024-key halves, fused with scoresT = KT.T @ QT and exp
    (no max subtraction; scores are O(1) so exp cannot overflow)
  - rowsum[q] via ones-row matmul over expT + 8 tiny transpose matmuls
  - OT [D, 1024] = V.T @ expT in two e-group passes (4 e-tiles x full PSUM)
  - y = (OT.T @ WprojT) * (1/rowsum) + bias

Matmuls run in float32r (fp32 storage, reduced-precision PE multiply,
1 cycle/row vs fp32's 4). Keys are permuted per-core (own half first) -
attention is permutation invariant.

SBUF tags are manually aliased across phases (pool memory is the static sum
over tags): w0..7 carry wv -> wq -> KT-half -> wp; xt0..7 carry xT -> V
reload; qt0..7 carry QT -> OT. PSUM uses explicit bank tags b0..b7.
"""

import numpy as np
from contextlib import ExitStack

D = 1024
S = 2048
SQ = 1024  # queries per core
P = 128
SCALE = float(1.0 / np.sqrt(np.float32(D)).astype(np.float32))

_CACHED = {}

COLSUM_F32R = True


def _build_nc():
    import concourse.bass as bass
    import concourse.tile as tile
    from concourse import bacc, mybir

    DT = mybir.dt.float32
    F32R = mybir.dt.float32r
    FP = mybir.dt.float32
    Exp = mybir.ActivationFunctionType.Exp
    MUL = mybir.AluOpType.mult
    ADD = mybir.AluOpType.add

    nc = bacc.Bacc("TRN2", target_bir_lowering=False)
    xt_d = nc.declare_dram_parameter("xt", [D, S], F32R, isOutput=False)
    wq_d = nc.declare_dram_parameter("wqt", [D, D], F32R, isOutput=False)
    wk_d = nc.declare_dram_parameter("wkt", [D, D], F32R, isOutput=False)
    wv_d = nc.declare_dram_parameter("wvt", [D, D], F32R, isOutput=False)
    wp_d = nc.declare_dram_parameter("wpt", [D, D], F32R, isOutput=False)
    bias_d = nc.declare_dram_parameter("bias", [P, D], DT, isOutput=False)
    ones_d = nc.declare_dram_parameter("ones", [P, 1], F32R, isOutput=False)
    y_d = nc.declare_dram_parameter("y", [SQ, D], DT, isOutput=True)

    ND = D // P     # 8 tiles along D
    NS = S // P     # 16 tiles along S

    with tile.TileContext(nc) as tc:
        with ExitStack() as ctx:
            pool = ctx.enter_context(tc.tile_pool(name="main", bufs=1))
            psum = ctx.enter_context(tc.tile_pool(name="psum", bufs=1, space="PSUM"))
            dram = ctx.enter_context(tc.tile_pool(name="dram", bufs=1, space="DRAM"))

            def ptile(shape, name, tag, bufs=1, dt=F32R):
                return pool.tile(shape, dt, name=name, tag=tag, bufs=bufs)

            def qbank(i, name, shape=(P, 1024)):
                # 4 PSUM tags x 2 banks each = all 8 banks
                return psum.tile(list(shape), FP, name=name, tag=f"q{i}", bufs=1)

            # ---- resident inputs (wv/xt interleaved so phase 1 starts early)
            wv = []
            xt = []
            for d in range(ND):
                t = ptile([P, D], f"w{d}", f"w{d}")
                nc.sync.dma_start(t[:], wv_d[d * P:(d + 1) * P, :])
                wv.append(t)
                t = ptile([P, S], f"xt{d}", f"xt{d}")
                nc.sync.dma_start(t[:], xt_d[d * P:(d + 1) * P, :])
                xt.append(t)
            ones_sb = ptile([P, 1], "ones", "ones")
            nc.sync.dma_start(ones_sb[:], ones_d[:, :])

            def load_wblk(src):
                blks = []
                for d in range(ND):
                    t = ptile([P, D], f"w{d}", f"w{d}")
                    nc.sync.dma_start(t[:], src[d * P:(d + 1) * P, :])
                    blks.append(t)
                return blks

            v_spill = dram.tile([S, D], F32R, name="vspill", tag="vspill")

            # ---- phase 1: V[sk][128, D] = sum_d xt[d][:, sk].T @ wv[d]; spill ----
            for sk in range(NS):
                pv = qbank(sk % 2, f"pv_{sk}")
                pv0, pv1 = pv[:, 0:512], pv[:, 512:1024]
                for d in range(ND):
                    lt = xt[d][:, sk * P:(sk + 1) * P]
                    nc.tensor.matmul(pv0, lt, wv[d][:, 0:512],
                                     start=(d == 0), stop=(d == ND - 1))
                    nc.tensor.matmul(pv1, lt, wv[d][:, 512:1024],
                                     start=(d == 0), stop=(d == ND - 1))
                vb = ptile([P, D], f"vstage{sk}", "vstage", bufs=2)
                nc.vector.tensor_copy(vb[:, 0:512], pv0)
                nc.vector.tensor_copy(vb[:, 512:1024], pv1)
                nc.sync.dma_start(v_spill[sk * P:(sk + 1) * P, :], vb[:])

            # ---- phase 2: QT[e][128, SQ] = sum_d wq[d][:, e].T @ xt[d][:, :SQ] ----
            wq = load_wblk(wq_d)
            qt = []
            for e in range(ND):
                qt.append(ptile([P, SQ], f"qt{e}", f"qt{e}"))
            for e in range(ND):
                pq = qbank(e % 2, f"pq_{e}")
                ps0, ps1 = pq[:, 0:512], pq[:, 512:1024]
                for d in range(ND):
                    lt = wq[d][:, e * P:(e + 1) * P]
                    nc.tensor.matmul(ps0, lt, xt[d][:, 0:512],
                                     start=(d == 0), stop=(d == ND - 1))
                    nc.tensor.matmul(ps1, lt, xt[d][:, 512:1024],
                                     start=(d == 0), stop=(d == ND - 1))
                nc.vector.tensor_copy(qt[e][:, 0:512], ps0)
                nc.vector.tensor_copy(qt[e][:, 512:1024], ps1)

            # ---- phase 3: KT in 1024-key halves (wk streamed as 128x128 tiles,
            #      each stationary reused over both 512-chunks of the half),
            #      fused with scoresT + exp; expT[sk][128, SQ] resident ----
            expT = []
            for sk in range(NS):
                expT.append(ptile([P, SQ], f"expT{sk}", f"expT{sk}"))
            for half in range(2):  # 1024-key half
                kth = []
                for e in range(ND):
                    pk = qbank(2 + e % 2, f"pk_{half}_{e}")
                    pk0, pk1 = pk[:, 0:512], pk[:, 512:1024]
                    for d in range(ND):
                        wkt = ptile([P, P], f"wks_{half}_{e}_{d}", "wks", bufs=6)
                        nc.sync.dma_start(
                            wkt[:], wk_d[d * P:(d + 1) * P, e * P:(e + 1) * P])
                        base = half * 1024
                        nc.tensor.matmul(pk0, wkt[:], xt[d][:, base:base + 512],
                                         start=(d == 0), stop=(d == ND - 1))
                        nc.tensor.matmul(pk1, wkt[:],
                                         xt[d][:, base + 512:base + 1024],
                                         start=(d == 0), stop=(d == ND - 1))
                    kt_sb = ptile([P, 1024], f"kth_{half}_{e}", f"w{e}")
                    nc.vector.tensor_copy(kt_sb[:, 0:512], pk0)
                    nc.vector.tensor_copy(kt_sb[:, 512:1024], pk1)
                    kth.append(kt_sb)
                for t in range(8):
                    sk = half * 8 + t
                    psc = qbank(t % 2, f"psc_{sk}")
                    ps0, ps1 = psc[:, 0:512], psc[:, 512:1024]
                    for e in range(ND):
                        lt = kth[e][:, t * P:(t + 1) * P]
                        nc.tensor.matmul(ps0, lt, qt[e][:, 0:512],
                                         start=(e == 0), stop=(e == ND - 1))
                        nc.tensor.matmul(ps1, lt, qt[e][:, 512:1024],
                                         start=(e == 0), stop=(e == ND - 1))
                    nc.scalar.activation(expT[sk][:, 0:512], ps0, Exp, scale=SCALE)
                    nc.scalar.activation(expT[sk][:, 512:1024], ps1, Exp, scale=SCALE)

            # ---- phase 4: rowsum -> r_sb [128, 8] (col j: 1/rowsum, q-tile j)
            # colsum: ones-row matmul over expT -> rs_row [1, 1024]
            pc = qbank(2, "pcs", shape=(1, 1024))
            pc0, pc1 = pc[0:1, 0:512], pc[0:1, 512:1024]
            for sk in range(NS):
                if COLSUM_F32R:
                    lt = ones_sb[:]
                    r0 = expT[sk][:, 0:512]
                    r1 = expT[sk][:, 512:1024]
                else:
                    lt = ones_sb[:].bitcast(DT)
                    r0 = expT[sk][:, 0:512].bitcast(DT)
                    r1 = expT[sk][:, 512:1024].bitcast(DT)
                nc.tensor.matmul(pc0, lt, r0, start=(sk == 0), stop=(sk == NS - 1))
                nc.tensor.matmul(pc1, lt, r1, start=(sk == 0), stop=(sk == NS - 1))
            rs_row = ptile([1, SQ], "rs_row", "rs_row", dt=DT)
            nc.vector.tensor_copy(rs_row[0:1, 0:512], pc0)
            nc.vector.tensor_copy(rs_row[0:1, 512:1024], pc1)
            # transpose [1, 1024] -> [128, 8] via 8 K=1 fp32 matmuls
            psr = qbank(3, "psr", shape=(P, ND))
            one1 = ones_sb[0:1, 0:1].bitcast(DT)
            for j in range(ND):
                nc.tensor.matmul(psr[:, j:j + 1], rs_row[0:1, j * P:(j + 1) * P],
                                 one1, start=True, stop=True)
            r_sb = ptile([P, ND], "rsb", "rsb", dt=DT)
            nc.vector.reciprocal(r_sb[:], psr[:])

            # ---- phase 5: OT[e][128, SQ] accumulated in PSUM, 2 e-group passes
            ot = [ptile([P, SQ], f"ot_sb{e}", f"qt{e}") for e in range(ND)]
            for eg in range(2):
                otp = [qbank(i, f"ot_{eg}_{i}") for i in range(4)]
                for sk in range(NS):
                    vb = ptile([P, D], f"vload_{eg}_{sk}", f"xt{sk % 3}")
                    nc.sync.dma_start(vb[:], v_spill[sk * P:(sk + 1) * P, :])
                    for i in range(4):
                        e = eg * 4 + i
                        lt = vb[:, e * P:(e + 1) * P]
                        nc.tensor.matmul(otp[i][:, 0:512], lt,
                                         expT[sk][:, 0:512],
                                         start=(sk == 0), stop=(sk == NS - 1))
                        nc.tensor.matmul(otp[i][:, 512:1024], lt,
                                         expT[sk][:, 512:1024],
                                         start=(sk == 0), stop=(sk == NS - 1))
                for i in range(4):
                    e = eg * 4 + i
                    nc.vector.tensor_copy(ot[e][:, 0:512], otp[i][:, 0:512])
                    nc.vector.tensor_copy(ot[e][:, 512:1024], otp[i][:, 512:1024])

            # ---- phase 6: proj y[g] = (OT.T @ wpT) * r + bias ----
            wp = load_wblk(wp_d)
            bias_sb = ptile([P, D], "bias", "vstage", bufs=2, dt=DT)
            nc.sync.dma_start(bias_sb[:], bias_d[:, :])
            for g in range(8):
                py = qbank(2 + g % 2, f"py_{g}")
                ps0, ps1 = py[:, 0:512], py[:, 512:1024]
                for e in range(ND):
                    lt = ot[e][:, g * P:(g + 1) * P]
                    nc.tensor.matmul(ps0, lt, wp[e][:, 0:512],
                                     start=(e == 0), stop=(e == ND - 1))
                    nc.tensor.matmul(ps1, lt, wp[e][:, 512:1024],
                                     start=(e == 0), stop=(e == ND - 1))
                ysb = ptile([P, D], f"ysb_{g}", "vstage", bufs=2, dt=DT)
                rcol = r_sb[:, g:g + 1]
                nc.vector.scalar_tensor_tensor(ysb[:, 0:512], ps0, rcol,
                                               bias_sb[:, 0:512], MUL, ADD)
                nc.vector.scalar_tensor_tensor(ysb[:, 512:1024], ps1, rcol,
                                               bias_sb[:, 512:1024], MUL, ADD)
                nc.sync.dma_start(y_d[g * P:(g + 1) * P, :], ysb[:])

    nc.compile()
    return nc


def _get_nc():
    if "nc" not in _CACHED:
        _CACHED["nc"] = _build_nc()
    return _CACHED["nc"]


def make_in_maps(x, w_qkv, w_proj, b_proj):
    wqT = np.ascontiguousarray(w_qkv[0:D].T)
    wkT = np.ascontiguousarray(w_qkv[D:2 * D].T)
    wvT = np.ascontiguousarray(w_qkv[2 * D:3 * D].T)
    wpT = np.ascontiguousarray(w_proj.T)
    bias = np.ascontiguousarray(np.broadcast_to(b_proj[None, :], (P, D)))
    ones = np.ones((P, 1), dtype=np.float32)
    in_maps = []
    for c in range(8):
        b, h = c // 2, c % 2
        own = x[b, h * SQ:(h + 1) * SQ]       # [1024, D] our queries
        other = x[b, (1 - h) * SQ:(2 - h) * SQ]
        xt = np.ascontiguousarray(np.concatenate([own.T, other.T], axis=1))
        in_maps.append({
            "xt": xt, "wqt": wqT, "wkt": wkT, "wvt": wvT, "wpt": wpT,
            "bias": bias, "ones": ones,
        })
    return in_maps


def gather_out(results):
    out = np.empty((4, S, D), dtype=np.float32)
    for c in range(8):
        b, h = c // 2, c % 2
        out[b, h * SQ:(h + 1) * SQ] = results[c]["y"]
    return out


def kernel(x, w_qkv, w_proj, b_proj):
    from concourse import bass_utils
    nc = _get_nc()
    in_maps = make_in_maps(np.asarray(x, dtype=np.float32),
                           np.asarray(w_qkv, dtype=np.float32),
                           np.asarray(w_proj, dtype=np.float32),
                           np.asarray(b_proj, dtype=np.float32))
    res = bass_utils.run_bass_kernel_spmd(nc, in_maps, list(range(8))).results
    return gather_out(res)


# revision 13
# speedup vs baseline: 4.0255x; 1.1990x over previous
"""Trainium2 Bass kernel for nn_AttentionBlock (B=4, S=2048, D=1024, single head).

Sharding: 8 cores = 4 batches x 2 query-halves; each core owns 1024 queries
of one batch and returns that [1024, 1024] slice of the output.

Algebraic restructure: scores = Q @ K.T = Xq Wq^T Wk Xk^T, so with the
weight-only fold W2 = Wk^T @ Wq (done host-side, x-independent):
    G      [D, 1024]  = W2 @ Xq^T           (device)
    scoresT[2048,1024] = X @ G   via lhsT = xT tiles (already resident)
Q and K are never materialized. Remaining phases:
    V [2048, D] = X @ Wv^T (spilled to DRAM, streamed back twice)
    expT = exp(scoresT * scale)  (no max subtraction; scores are O(1))
    rowsum via ones-row matmul + 8 tiny transpose matmuls; r = 1/rowsum
    OT [D, 1024] = V.T @ expT in two e-group passes (4 e-tiles = all PSUM)
    y = (OT.T @ WprojT) * r + bias

Matmuls run in float32r (fp32 storage, reduced-precision PE multiply,
1 cycle/row vs fp32's 4). Keys are permuted per-core (own half first) -
attention is permutation invariant. Walrus is invoked with
--enable-ldw-opt=true so LDWEIGHTS overlaps in-flight matmuls.

SBUF tags are manually aliased across phases (pool memory is the static sum
over tags): w0..7 carry wv -> W2^T -> wp; xt0..7 carry xT -> V reload;
g0..7 carry G -> OT. PSUM uses 4 double-bank tags q0..q3.
"""

import numpy as np
from contextlib import ExitStack

D = 1024
S = 2048
SQ = 1024  # queries per core
P = 128
SCALE = float(1.0 / np.sqrt(np.float32(D)).astype(np.float32))

_CACHED = {}


def _patch_walrus_flags():
    from concourse import bass_utils
    if getattr(bass_utils, "_ldw_opt_patched", False):
        return
    orig = bass_utils.run_command

    def run_command(cmd, **kw):
        cmd = ["--enable-ldw-opt=true" if c == "--enable-ldw-opt=false" else c
               for c in cmd]
        return orig(cmd, **kw)

    bass_utils.run_command = run_command
    bass_utils._ldw_opt_patched = True


def _build_nc():
    import concourse.bass as bass
    import concourse.tile as tile
    from concourse import bacc, mybir

    DT = mybir.dt.float32
    F32R = mybir.dt.float32r
    FP = mybir.dt.float32
    Exp = mybir.ActivationFunctionType.Exp
    MUL = mybir.AluOpType.mult
    ADD = mybir.AluOpType.add

    nc = bacc.Bacc("TRN2", target_bir_lowering=False)
    xt_d = nc.declare_dram_parameter("xt", [D, S], F32R, isOutput=False)
    w2t_d = nc.declare_dram_parameter("w2t", [D, D], F32R, isOutput=False)
    wv_d = nc.declare_dram_parameter("wvt", [D, D], F32R, isOutput=False)
    wp_d = nc.declare_dram_parameter("wpt", [D, D], F32R, isOutput=False)
    bias_d = nc.declare_dram_parameter("bias", [P, D], DT, isOutput=False)
    ones_d = nc.declare_dram_parameter("ones", [P, 1], F32R, isOutput=False)
    y_d = nc.declare_dram_parameter("y", [SQ, D], DT, isOutput=True)

    ND = D // P     # 8 tiles along D
    NS = S // P     # 16 tiles along S

    with tile.TileContext(nc) as tc:
        with ExitStack() as ctx:
            pool = ctx.enter_context(tc.tile_pool(name="main", bufs=1))
            psum = ctx.enter_context(tc.tile_pool(name="psum", bufs=1, space="PSUM"))
            dram = ctx.enter_context(tc.tile_pool(name="dram", bufs=1, space="DRAM"))

            def ptile(shape, name, tag, bufs=1, dt=F32R):
                return pool.tile(shape, dt, name=name, tag=tag, bufs=bufs)

            def qbank(i, name, shape=(P, 1024)):
                # 4 PSUM tags x 2 banks each = all 8 banks
                return psum.tile(list(shape), FP, name=name, tag=f"q{i}", bufs=1)

            # ---- resident inputs (wv/xt interleaved so phase 1 starts early)
            wv = []
            xt = []
            for d in range(ND):
                t = ptile([P, D], f"w{d}", f"w{d}")
                nc.sync.dma_start(t[:], wv_d[d * P:(d + 1) * P, :])
                wv.append(t)
                t = ptile([P, S], f"xt{d}", f"xt{d}")
                nc.sync.dma_start(t[:], xt_d[d * P:(d + 1) * P, :])
                xt.append(t)
            ones_sb = ptile([P, 1], "ones", "ones")
            nc.sync.dma_start(ones_sb[:], ones_d[:, :])

            def load_wblk(src, tagfn=lambda d: f"w{d}"):
                blks = []
                for d in range(ND):
                    tg = tagfn(d)
                    t = ptile([P, D], f"{tg}_ld", tg)
                    nc.sync.dma_start(t[:], src[d * P:(d + 1) * P, :])
                    blks.append(t)
                return blks

            v_spill = dram.tile([S, D], F32R, name="vspill", tag="vspill")

            # ---- phase 1: V[sk][128, D] = sum_d xt[d][:, sk].T @ wv[d]; spill ----
            for sk in range(NS):
                pv = qbank(sk % 4, f"pv_{sk}")
                pv0, pv1 = pv[:, 0:512], pv[:, 512:1024]
                for d in range(ND):
                    lt = xt[d][:, sk * P:(sk + 1) * P]
                    nc.tensor.matmul(pv0, lt, wv[d][:, 0:512],
                                     start=(d == 0), stop=(d == ND - 1))
                    nc.tensor.matmul(pv1, lt, wv[d][:, 512:1024],
                                     start=(d == 0), stop=(d == ND - 1))
                vb = ptile([P, D], f"vstage{sk}", "vstage", bufs=2)
                nc.vector.tensor_copy(vb[:, 0:512], pv0)
                nc.vector.tensor_copy(vb[:, 512:1024], pv1)
                nc.sync.dma_start(v_spill[sk * P:(sk + 1) * P, :], vb[:])

            # ---- phase 2: G[g][128, SQ] = sum_d w2t[d][:, g].T @ xt[d][:, :SQ]
            w2t = load_wblk(w2t_d, tagfn=lambda d: f"expT{8 + d}")
            g_sb = []
            for g in range(ND):
                g_sb.append(ptile([P, SQ], f"g{g}", f"g{g}"))
            for g in range(ND):
                pg = qbank(g % 4, f"pg_{g}")
                ps0, ps1 = pg[:, 0:512], pg[:, 512:1024]
                for d in range(ND):
                    lt = w2t[d][:, g * P:(g + 1) * P]
                    nc.tensor.matmul(ps0, lt, xt[d][:, 0:512],
                                     start=(d == 0), stop=(d == ND - 1))
                    nc.tensor.matmul(ps1, lt, xt[d][:, 512:1024],
                                     start=(d == 0), stop=(d == ND - 1))
                nc.vector.tensor_copy(g_sb[g][:, 0:512], ps0)
                nc.vector.tensor_copy(g_sb[g][:, 512:1024], ps1)

            # ---- phase 3: scoresT[sk][128, SQ] = sum_d xt[d][:, sk].T @ G[d],
            #      exp fused on ScalarE; expT[sk][128, SQ] resident ----
            expT = []
            for sk in range(NS):
                expT.append(ptile([P, SQ], f"expT{sk}", f"expT{sk}"))
            for sk in range(NS):
                psc = qbank(sk % 4, f"psc_{sk}")
                ps0, ps1 = psc[:, 0:512], psc[:, 512:1024]
                for d in range(ND):
                    lt = xt[d][:, sk * P:(sk + 1) * P]
                    nc.tensor.matmul(ps0, lt, g_sb[d][:, 0:512],
                                     start=(d == 0), stop=(d == ND - 1))
                    nc.tensor.matmul(ps1, lt, g_sb[d][:, 512:1024],
                                     start=(d == 0), stop=(d == ND - 1))
                nc.scalar.activation(expT[sk][:, 0:512], ps0, Exp, scale=SCALE)
                nc.scalar.activation(expT[sk][:, 512:1024], ps1, Exp, scale=SCALE)

            # ---- phase 4: rowsum -> r_sb [128, 8] (col j: 1/rowsum, q-tile j)
            pc = qbank(0, "pcs", shape=(1, 1024))
            pc0, pc1 = pc[0:1, 0:512], pc[0:1, 512:1024]
            for sk in range(NS):
                nc.tensor.matmul(pc0, ones_sb[:], expT[sk][:, 0:512],
                                 start=(sk == 0), stop=(sk == NS - 1))
                nc.tensor.matmul(pc1, ones_sb[:], expT[sk][:, 512:1024],
                                 start=(sk == 0), stop=(sk == NS - 1))
            rs_row = ptile([1, SQ], "rs_row", "rs_row", dt=DT)
            nc.vector.tensor_copy(rs_row[0:1, 0:512], pc0)
            nc.vector.tensor_copy(rs_row[0:1, 512:1024], pc1)
            # transpose [1, 1024] -> [128, 8] via 8 K=1 fp32 matmuls
            psr = qbank(1, "psr", shape=(P, ND))
            one1 = ones_sb[0:1, 0:1].bitcast(DT)
            for j in range(ND):
                nc.tensor.matmul(psr[:, j:j + 1], rs_row[0:1, j * P:(j + 1) * P],
                                 one1, start=True, stop=True)
            r_sb = ptile([P, ND], "rsb", "rsb", dt=DT)
            nc.vector.reciprocal(r_sb[:], psr[:])

            # ---- phase 5: OT[e][128, SQ] accumulated in PSUM, 2 e-group passes
            ot = [ptile([P, SQ], f"ot_sb{e}", f"g{e}") for e in range(ND)]
            for eg in range(2):
                otp = [qbank(i, f"ot_{eg}_{i}") for i in range(4)]
                for sk in range(NS):
                    vb = ptile([P, D], f"vload_{eg}_{sk}", f"xt{sk % 3}")
                    nc.sync.dma_start(vb[:], v_spill[sk * P:(sk + 1) * P, :])
                    for i in range(4):
                        e = eg * 4 + i
                        lt = vb[:, e * P:(e + 1) * P]
                        nc.tensor.matmul(otp[i][:, 0:512], lt,
                                         expT[sk][:, 0:512],
                                         start=(sk == 0), stop=(sk == NS - 1))
                        nc.tensor.matmul(otp[i][:, 512:1024], lt,
                                         expT[sk][:, 512:1024],
                                         start=(sk == 0), stop=(sk == NS - 1))
                for i in range(4):
                    e = eg * 4 + i
                    nc.vector.tensor_copy(ot[e][:, 0:512], otp[i][:, 0:512])
                    nc.vector.tensor_copy(ot[e][:, 512:1024], otp[i][:, 512:1024])

            # ---- phase 6: proj y[g] = (OT.T @ wpT) * r + bias ----
            wp = load_wblk(wp_d)
            bias_sb = ptile([P, D], "bias", "vstage", bufs=2, dt=DT)
            nc.sync.dma_start(bias_sb[:], bias_d[:, :])
            for g in range(8):
                py = qbank(g % 4, f"py_{g}")
                ps0, ps1 = py[:, 0:512], py[:, 512:1024]
                for e in range(ND):
                    lt = ot[e][:, g * P:(g + 1) * P]
                    nc.tensor.matmul(ps0, lt, wp[e][:, 0:512],
                                     start=(e == 0), stop=(e == ND - 1))
                    nc.tensor.matmul(ps1, lt, wp[e][:, 512:1024],
                                     start=(e == 0), stop=(e == ND - 1))
                ysb = ptile([P, D], f"ysb_{g}", "vstage", bufs=2, dt=DT)
                rcol = r_sb[:, g:g + 1]
                nc.vector.scalar_tensor_tensor(ysb[:, 0:512], ps0, rcol,
                                               bias_sb[:, 0:512], MUL, ADD)
                nc.vector.scalar_tensor_tensor(ysb[:, 512:1024], ps1, rcol,
                                               bias_sb[:, 512:1024], MUL, ADD)
                nc.sync.dma_start(y_d[g * P:(g + 1) * P, :], ysb[:])

    nc.compile()
    return nc


def _get_nc():
    if "nc" not in _CACHED:
        _CACHED["nc"] = _build_nc()
    return _CACHED["nc"]


def make_in_maps(x, w_qkv, w_proj, b_proj):
    wq = w_qkv[0:D]
    wk = w_qkv[D:2 * D]
    w2 = wk.T @ wq                       # weight-only fold: scores = X W2 Xq^T
    w2T = np.ascontiguousarray(w2.T)
    wvT = np.ascontiguousarray(w_qkv[2 * D:3 * D].T)
    wpT = np.ascontiguousarray(w_proj.T)
    bias = np.ascontiguousarray(np.broadcast_to(b_proj[None, :], (P, D)))
    ones = np.ones((P, 1), dtype=np.float32)
    in_maps = []
    for c in range(8):
        b, h = c // 2, c % 2
        own = x[b, h * SQ:(h + 1) * SQ]       # [1024, D] our queries
        other = x[b, (1 - h) * SQ:(2 - h) * SQ]
        xt = np.ascontiguousarray(np.concatenate([own.T, other.T], axis=1))
        in_maps.append({
            "xt": xt, "w2t": w2T, "wvt": wvT, "wpt": wpT,
            "bias": bias, "ones": ones,
        })
    return in_maps


def gather_out(results):
    out = np.empty((4, S, D), dtype=np.float32)
    for c in range(8):
        b, h = c // 2, c % 2
        out[b, h * SQ:(h + 1) * SQ] = results[c]["y"]
    return out


def kernel(x, w_qkv, w_proj, b_proj):
    from concourse import bass_utils
    _patch_walrus_flags()
    nc = _get_nc()
    in_maps = make_in_maps(np.asarray(x, dtype=np.float32),
                           np.asarray(w_qkv, dtype=np.float32),
                           np.asarray(w_proj, dtype=np.float32),
                           np.asarray(b_proj, dtype=np.float32))
    res = bass_utils.run_bass_kernel_spmd(nc, in_maps, list(range(8))).results
    return gather_out(res)


# revision 14
# speedup vs baseline: 4.4210x; 1.0983x over previous
"""Trainium2 Bass kernel for nn_AttentionBlock (B=4, S=2048, D=1024, single head).

Sharding: 8 cores = 4 batches x 2 query-halves; each core owns 1024 queries
of one batch and returns that [1024, 1024] slice of the output.

Algebraic restructure: scores = Q @ K.T = Xq Wq^T Wk Xk^T, so with the
weight-only fold W2 = Wk^T @ Wq (done host-side, x-independent):
    G      [D, 1024]  = W2 @ Xq^T           (device)
    scoresT[2048,1024] = X @ G   via lhsT = xT tiles (already resident)
Q and K are never materialized. Remaining phases:
    V [2048, D] = X @ Wv^T (spilled to DRAM, streamed back twice)
    expT = exp(scoresT * scale)  (no max subtraction; scores are O(1))
    rowsum via ones-row matmul + 8 tiny transpose matmuls; r = 1/rowsum
    OT [D, 1024] = V.T @ expT in two e-group passes (4 e-tiles = all PSUM)
    y = (OT.T @ WprojT) * r + bias

Matmuls run in float32r (fp32 storage, reduced-precision PE multiply,
1 cycle/row vs fp32's 4). Keys are permuted per-core (own half first) -
attention is permutation invariant. Walrus is invoked with
--enable-ldw-opt=true so LDWEIGHTS overlaps in-flight matmuls.

SBUF tags are manually aliased across phases (pool memory is the static sum
over tags): w0..7 carry wv -> W2^T -> wp; xt0..7 carry xT -> V reload;
g0..7 carry G -> OT. PSUM uses 4 double-bank tags q0..q3.
"""

import numpy as np
from contextlib import ExitStack

D = 1024
S = 2048
SQ = 1024  # queries per core
P = 128
SCALE = float(1.0 / np.sqrt(np.float32(D)).astype(np.float32))

_CACHED = {}


def _patch_walrus_flags():
    from concourse import bass_utils
    if getattr(bass_utils, "_ldw_opt_patched", False):
        return
    orig = bass_utils.run_command

    def run_command(cmd, **kw):
        cmd = ["--enable-ldw-opt=true" if c == "--enable-ldw-opt=false" else c
               for c in cmd]
        return orig(cmd, **kw)

    bass_utils.run_command = run_command
    bass_utils._ldw_opt_patched = True


def _build_nc():
    import concourse.bass as bass
    import concourse.tile as tile
    from concourse import bacc, mybir

    DT = mybir.dt.float32
    F32R = mybir.dt.float32r
    FP = mybir.dt.float32
    Exp = mybir.ActivationFunctionType.Exp
    MUL = mybir.AluOpType.mult
    ADD = mybir.AluOpType.add

    nc = bacc.Bacc("TRN2", target_bir_lowering=False)
    xt_d = nc.declare_dram_parameter("xt", [D, S], F32R, isOutput=False)
    w2t_d = nc.declare_dram_parameter("w2t", [D, D], F32R, isOutput=False)
    wv_d = nc.declare_dram_parameter("wvt", [D, D], F32R, isOutput=False)
    wp_d = nc.declare_dram_parameter("wpt", [D, D], F32R, isOutput=False)
    bias_d = nc.declare_dram_parameter("bias", [P, D], DT, isOutput=False)
    ones_d = nc.declare_dram_parameter("ones", [P, 1], F32R, isOutput=False)
    y_d = nc.declare_dram_parameter("y", [SQ, D], DT, isOutput=True)

    ND = D // P     # 8 tiles along D
    NS = S // P     # 16 tiles along S

    with tile.TileContext(nc) as tc:
        with ExitStack() as ctx:
            pool = ctx.enter_context(tc.tile_pool(name="main", bufs=1))
            psum = ctx.enter_context(tc.tile_pool(name="psum", bufs=1, space="PSUM"))
            dram = ctx.enter_context(tc.tile_pool(name="dram", bufs=1, space="DRAM"))

            def ptile(shape, name, tag, bufs=1, dt=F32R):
                return pool.tile(shape, dt, name=name, tag=tag, bufs=bufs)

            def qbank(i, name, shape=(P, 1024)):
                # 4 PSUM tags x 2 banks each = all 8 banks
                return psum.tile(list(shape), FP, name=name, tag=f"q{i}", bufs=1)

            # ---- resident inputs (wv/xt interleaved so phase 1 starts early)
            wv = []
            xt = []
            for d in range(ND):
                t = ptile([P, D], f"w{d}", f"w{d}")
                nc.sync.dma_start(t[:], wv_d[d * P:(d + 1) * P, :])
                wv.append(t)
                t = ptile([P, S], f"xt{d}", f"xt{d}")
                nc.sync.dma_start(t[:], xt_d[d * P:(d + 1) * P, :])
                xt.append(t)
            ones_sb = ptile([P, 1], "ones", "ones")
            nc.sync.dma_start(ones_sb[:], ones_d[:, :])

            def load_wblk(src, tagfn=lambda d: f"w{d}"):
                blks = []
                for d in range(ND):
                    tg = tagfn(d)
                    t = ptile([P, D], f"{tg}_ld", tg)
                    nc.sync.dma_start(t[:], src[d * P:(d + 1) * P, :])
                    blks.append(t)
                return blks

            v_spill = dram.tile([S, D], F32R, name="vspill", tag="vspill")

            # ---- phase 1: V[sk][128, D] = sum_d xt[d][:, sk].T @ wv[d]; spill ----
            for sk in range(NS):
                pv = qbank(sk % 4, f"pv_{sk}")
                pv0, pv1 = pv[:, 0:512], pv[:, 512:1024]
                for d in range(ND):
                    lt = xt[d][:, sk * P:(sk + 1) * P]
                    nc.tensor.matmul(pv0, lt, wv[d][:, 0:512],
                                     start=(d == 0), stop=(d == ND - 1))
                    nc.tensor.matmul(pv1, lt, wv[d][:, 512:1024],
                                     start=(d == 0), stop=(d == ND - 1))
                vb = ptile([P, D], f"vstage{sk}", "vstage", bufs=2)
                nc.vector.tensor_copy(vb[:, 0:512], pv0)
                nc.vector.tensor_copy(vb[:, 512:1024], pv1)
                nc.sync.dma_start(v_spill[sk * P:(sk + 1) * P, :], vb[:])

            # ---- phase 2: G[g][128, SQ] = sum_d w2t[d][:, g].T @ xt[d][:, :SQ]
            w2t = load_wblk(w2t_d, tagfn=lambda d: f"expT{8 + d}")
            g_sb = []
            for g in range(ND):
                g_sb.append(ptile([P, SQ], f"g{g}", f"g{g}"))
            for g in range(ND):
                pg = qbank(g % 4, f"pg_{g}")
                ps0, ps1 = pg[:, 0:512], pg[:, 512:1024]
                for d in range(ND):
                    lt = w2t[d][:, g * P:(g + 1) * P]
                    nc.tensor.matmul(ps0, lt, xt[d][:, 0:512],
                                     start=(d == 0), stop=(d == ND - 1))
                    nc.tensor.matmul(ps1, lt, xt[d][:, 512:1024],
                                     start=(d == 0), stop=(d == ND - 1))
                nc.vector.tensor_copy(g_sb[g][:, 0:512], ps0)
                nc.vector.tensor_copy(g_sb[g][:, 512:1024], ps1)

            # ---- phase 3: scoresT[sk][128, SQ] = sum_d xt[d][:, sk].T @ G[d],
            #      exp fused on ScalarE; expT[sk][128, SQ] resident ----
            expT = []
            for sk in range(NS):
                expT.append(ptile([P, SQ], f"expT{sk}", f"expT{sk}"))
            for sk in range(NS):
                psc = qbank(sk % 4, f"psc_{sk}")
                ps0, ps1 = psc[:, 0:512], psc[:, 512:1024]
                for d in range(ND):
                    lt = xt[d][:, sk * P:(sk + 1) * P]
                    nc.tensor.matmul(ps0, lt, g_sb[d][:, 0:512],
                                     start=(d == 0), stop=(d == ND - 1))
                    nc.tensor.matmul(ps1, lt, g_sb[d][:, 512:1024],
                                     start=(d == 0), stop=(d == ND - 1))
                nc.scalar.activation(expT[sk][:, 0:512], ps0, Exp, scale=SCALE)
                nc.scalar.activation(expT[sk][:, 512:1024], ps1, Exp, scale=SCALE)

            # ---- phase 4: rowsum -> r_sb [128, 8] (col j: 1/rowsum, q-tile j)
            pc = qbank(0, "pcs", shape=(1, 1024))
            pc0, pc1 = pc[0:1, 0:512], pc[0:1, 512:1024]
            for sk in range(NS):
                nc.tensor.matmul(pc0, ones_sb[:], expT[sk][:, 0:512],
                                 start=(sk == 0), stop=(sk == NS - 1))
                nc.tensor.matmul(pc1, ones_sb[:], expT[sk][:, 512:1024],
                                 start=(sk == 0), stop=(sk == NS - 1))
            rs_row = ptile([1, SQ], "rs_row", "rs_row", dt=DT)
            nc.vector.tensor_copy(rs_row[0:1, 0:512], pc0)
            nc.vector.tensor_copy(rs_row[0:1, 512:1024], pc1)
            # transpose [1, 1024] -> [128, 8] via 8 K=1 fp32 matmuls
            psr = qbank(1, "psr", shape=(P, ND))
            one1 = ones_sb[0:1, 0:1].bitcast(DT)
            for j in range(ND):
                nc.tensor.matmul(psr[:, j:j + 1], rs_row[0:1, j * P:(j + 1) * P],
                                 one1, start=True, stop=True)
            r_sb = ptile([P, ND], "rsb", "rsb", dt=DT)
            nc.vector.reciprocal(r_sb[:], psr[:])

            # ---- phase 5: OT[e][128, SQ] accumulated in PSUM, 2 e-group passes
            ot = [ptile([P, SQ], f"ot_sb{e}", f"g{e}") for e in range(ND)]
            for eg in range(2):
                otp = [qbank(i, f"ot_{eg}_{i}") for i in range(4)]
                for sk in range(NS):
                    vb = ptile([P, D], f"vload_{eg}_{sk}", f"xt{sk % 3}")
                    nc.sync.dma_start(vb[:], v_spill[sk * P:(sk + 1) * P, :])
                    for i in range(4):
                        e = eg * 4 + i
                        lt = vb[:, e * P:(e + 1) * P]
                        nc.tensor.matmul(otp[i][:, 0:512], lt,
                                         expT[sk][:, 0:512],
                                         start=(sk == 0), stop=(sk == NS - 1))
                        nc.tensor.matmul(otp[i][:, 512:1024], lt,
                                         expT[sk][:, 512:1024],
                                         start=(sk == 0), stop=(sk == NS - 1))
                for i in range(4):
                    e = eg * 4 + i
                    nc.vector.tensor_copy(ot[e][:, 0:512], otp[i][:, 0:512])
                    nc.vector.tensor_copy(ot[e][:, 512:1024], otp[i][:, 512:1024])

            # ---- phase 6: proj y[g] = (OT.T @ wpT) * r + bias ----
            wp = load_wblk(wp_d)
            bias_sb = ptile([P, D], "bias", "vstage", bufs=2, dt=DT)
            nc.sync.dma_start(bias_sb[:], bias_d[:, :])
            for g in range(8):
                py = qbank(g % 4, f"py_{g}")
                ps0, ps1 = py[:, 0:512], py[:, 512:1024]
                for e in range(ND):
                    lt = ot[e][:, g * P:(g + 1) * P]
                    nc.tensor.matmul(ps0, lt, wp[e][:, 0:512],
                                     start=(e == 0), stop=(e == ND - 1))
                    nc.tensor.matmul(ps1, lt, wp[e][:, 512:1024],
                                     start=(e == 0), stop=(e == ND - 1))
                ysb = ptile([P, D], f"ysb_{g}", "vstage", bufs=2, dt=DT)
                rcol = r_sb[:, g:g + 1]
                nc.vector.scalar_tensor_tensor(ysb[:, 0:512], ps0, rcol,
                                               bias_sb[:, 0:512], MUL, ADD)
                nc.vector.scalar_tensor_tensor(ysb[:, 512:1024], ps1, rcol,
                                               bias_sb[:, 512:1024], MUL, ADD)
                nc.sync.dma_start(y_d[g * P:(g + 1) * P, :], ysb[:])

    nc.compile()
    return nc


def _get_nc():
    if "nc" not in _CACHED:
        _CACHED["nc"] = _build_nc()
    return _CACHED["nc"]


def make_in_maps(x, w_qkv, w_proj, b_proj):
    wq = w_qkv[0:D]
    wk = w_qkv[D:2 * D]
    w2 = wk.T @ wq                       # weight-only fold: scores = X W2 Xq^T
    w2T = np.ascontiguousarray(w2.T)
    wvT = np.ascontiguousarray(w_qkv[2 * D:3 * D].T)
    wpT = np.ascontiguousarray(w_proj.T)
    bias = np.ascontiguousarray(np.broadcast_to(b_proj[None, :], (P, D)))
    ones = np.ones((P, 1), dtype=np.float32)
    in_maps = []
    for c in range(8):
        b, h = c // 2, c % 2
        own = x[b, h * SQ:(h + 1) * SQ]       # [1024, D] our queries
        other = x[b, (1 - h) * SQ:(2 - h) * SQ]
        xt = np.ascontiguousarray(np.concatenate([own.T, other.T], axis=1))
        in_maps.append({
            "xt": xt, "w2t": w2T, "wvt": wvT, "wpt": wpT,
            "bias": bias, "ones": ones,
        })
    return in_maps


def gather_out(results):
    out = np.empty((4, S, D), dtype=np.float32)
    for c in range(8):
        b, h = c // 2, c % 2
        out[b, h * SQ:(h + 1) * SQ] = results[c]["y"]
    return out


def kernel(x, w_qkv, w_proj, b_proj):
    from concourse import bass_utils
    nc = _get_nc()
    in_maps = make_in_maps(np.asarray(x, dtype=np.float32),
                           np.asarray(w_qkv, dtype=np.float32),
                           np.asarray(w_proj, dtype=np.float32),
                           np.asarray(b_proj, dtype=np.float32))
    res = bass_utils.run_bass_kernel_spmd(nc, in_maps, list(range(8))).results
    return gather_out(res)


# revision 16
# speedup vs baseline: 5.1131x; 1.1566x over previous
"""Trainium2 Bass kernel for nn_AttentionBlock (B=4, S=2048, D=1024, single head).

Sharding: 8 cores = 4 batches x 2 query-halves; each core owns 1024 queries
of one batch and returns that [1024, 1024] slice of the output.

Algebraic restructure: scores = Q @ K.T = Xq Wq^T Wk Xk^T, so with the
weight-only fold W2 = Wk^T @ Wq (done host-side, x-independent):
    G      [D, 1024]  = W2 @ Xq^T           (device)
    scoresT[2048,1024] = X @ G   via lhsT = xT tiles (already resident)
Q and K are never materialized. Remaining phases:
    V [2048, D] = X @ Wv^T (spilled to DRAM, streamed back twice)
    expT = exp(scoresT * scale)  (no max subtraction; scores are O(1))
    rowsum via ones-row matmul + 8 tiny transpose matmuls; r = 1/rowsum
    OT [D, 1024] = V.T @ expT in two e-group passes (4 e-tiles = all PSUM)
    y = (OT.T @ WprojT) * r + bias

Matmuls run in float32r (fp32 storage, reduced-precision PE multiply,
1 cycle/row vs fp32's 4). Keys are permuted per-core (own half first) -
attention is permutation invariant. Walrus is invoked with
--enable-ldw-opt=true so LDWEIGHTS overlaps in-flight matmuls.

SBUF tags are manually aliased across phases (pool memory is the static sum
over tags): w0..7 carry wv -> W2^T -> wp; xt0..7 carry xT -> V reload;
g0..7 carry G -> OT. PSUM uses 4 double-bank tags q0..q3.
"""

import numpy as np
from contextlib import ExitStack

D = 1024
S = 2048
SQ = 1024  # queries per core
P = 128
SCALE = float(1.0 / np.sqrt(np.float32(D)).astype(np.float32))

_CACHED = {}


def _patch_walrus_flags():
    from concourse import bass_utils
    if getattr(bass_utils, "_ldw_opt_patched", False):
        return
    orig = bass_utils.run_command

    def run_command(cmd, **kw):
        cmd = ["--enable-ldw-opt=true" if c == "--enable-ldw-opt=false" else c
               for c in cmd]
        return orig(cmd, **kw)

    bass_utils.run_command = run_command
    bass_utils._ldw_opt_patched = True


def _build_nc():
    import concourse.bass as bass
    import concourse.tile as tile
    from concourse import bacc, mybir

    DT = mybir.dt.float32
    F32R = mybir.dt.float32r
    FP = mybir.dt.float32
    Exp = mybir.ActivationFunctionType.Exp
    MUL = mybir.AluOpType.mult
    ADD = mybir.AluOpType.add

    nc = bacc.Bacc("TRN2", target_bir_lowering=False)
    xt_d = nc.declare_dram_parameter("xt", [D, S], F32R, isOutput=False)
    w2t_d = nc.declare_dram_parameter("w2t", [D, D], F32R, isOutput=False)
    wv_d = nc.declare_dram_parameter("wvt", [D, D], F32R, isOutput=False)
    wp_d = nc.declare_dram_parameter("wpt", [D, D], F32R, isOutput=False)
    bias_d = nc.declare_dram_parameter("bias", [P, D], DT, isOutput=False)
    ones_d = nc.declare_dram_parameter("ones", [P, 1], F32R, isOutput=False)
    y_d = nc.declare_dram_parameter("y", [SQ, D], DT, isOutput=True)

    ND = D // P     # 8 tiles along D
    NS = S // P     # 16 tiles along S

    with tile.TileContext(nc) as tc:
        with ExitStack() as ctx:
            pool = ctx.enter_context(tc.tile_pool(name="main", bufs=1))
            psum = ctx.enter_context(tc.tile_pool(name="psum", bufs=1, space="PSUM"))
            dram = ctx.enter_context(tc.tile_pool(name="dram", bufs=1, space="DRAM"))

            def ptile(shape, name, tag, bufs=1, dt=F32R):
                return pool.tile(shape, dt, name=name, tag=tag, bufs=bufs)

            def qbank(i, name, shape=(P, 1024)):
                # 4 PSUM tags x 2 banks each = all 8 banks
                return psum.tile(list(shape), FP, name=name, tag=f"q{i}", bufs=1)

            # ---- resident inputs (wv/xt interleaved so phase 1 starts early)
            wv = []
            xt = []
            for d in range(ND):
                t = ptile([P, D], f"w{d}", f"w{d}")
                nc.sync.dma_start(t[:], wv_d[d * P:(d + 1) * P, :])
                wv.append(t)
                t = ptile([P, S], f"xt{d}", f"xt{d}")
                nc.sync.dma_start(t[:, 0:1024], xt_d[d * P:(d + 1) * P, 0:1024])
                xt.append(t)
            ones_sb = ptile([P, 1], "ones", "ones")
            nc.sync.dma_start(ones_sb[:], ones_d[:, :])
            for d in range(ND):
                nc.sync.dma_start(xt[d][:, 1024:2048],
                                  xt_d[d * P:(d + 1) * P, 1024:2048])

            def load_wblk(src, tagfn=lambda d: f"w{d}"):
                blks = []
                for d in range(ND):
                    tg = tagfn(d)
                    t = ptile([P, D], f"{tg}_ld", tg)
                    nc.sync.dma_start(t[:], src[d * P:(d + 1) * P, :])
                    blks.append(t)
                return blks

            v_spill = dram.tile([S, D], F32R, name="vspill", tag="vspill")

            # ---- phase 1: V[sk][128, D] = sum_d xt[d][:, sk].T @ wv[d]; spill ----
            for sk in range(NS):
                pv = qbank(sk % 4, f"pv_{sk}")
                pv0, pv1 = pv[:, 0:512], pv[:, 512:1024]
                for d in range(ND):
                    lt = xt[d][:, sk * P:(sk + 1) * P]
                    nc.tensor.matmul(pv0, lt, wv[d][:, 0:512],
                                     start=(d == 0), stop=(d == ND - 1))
                    nc.tensor.matmul(pv1, lt, wv[d][:, 512:1024],
                                     start=(d == 0), stop=(d == ND - 1))
                vb = ptile([P, D], f"vstage{sk}", "vstage", bufs=3)
                nc.vector.tensor_copy(vb[:, 0:512], pv0)
                nc.vector.tensor_copy(vb[:, 512:1024], pv1)
                nc.sync.dma_start(v_spill[sk * P:(sk + 1) * P, :], vb[:])

            # ---- phase 2: G[g][128, SQ] = sum_d w2t[d][:, g].T @ xt[d][:, :SQ]
            w2t = load_wblk(w2t_d, tagfn=lambda d: f"expT{8 + d}")
            g_sb = []
            for g in range(ND):
                g_sb.append(ptile([P, SQ], f"g{g}", f"g{g}"))
            for g in range(ND):
                pg = qbank(g % 4, f"pg_{g}")
                ps0, ps1 = pg[:, 0:512], pg[:, 512:1024]
                for d in range(ND):
                    lt = w2t[d][:, g * P:(g + 1) * P]
                    nc.tensor.matmul(ps0, lt, xt[d][:, 0:512],
                                     start=(d == 0), stop=(d == ND - 1))
                    nc.tensor.matmul(ps1, lt, xt[d][:, 512:1024],
                                     start=(d == 0), stop=(d == ND - 1))
                nc.vector.tensor_copy(g_sb[g][:, 0:512], ps0)
                nc.vector.tensor_copy(g_sb[g][:, 512:1024], ps1)

            # ---- phase 3: scoresT[sk][128, SQ] = sum_d xt[d][:, sk].T @ G[d],
            #      exp fused on ScalarE; expT[sk][128, SQ] resident ----
            expT = []
            for sk in range(NS):
                expT.append(ptile([P, SQ], f"expT{sk}", f"expT{sk}"))
            for sk in range(NS):
                psc = qbank(sk % 4, f"psc_{sk}")
                ps0, ps1 = psc[:, 0:512], psc[:, 512:1024]
                for d in range(ND):
                    lt = xt[d][:, sk * P:(sk + 1) * P]
                    nc.tensor.matmul(ps0, lt, g_sb[d][:, 0:512],
                                     start=(d == 0), stop=(d == ND - 1))
                    nc.tensor.matmul(ps1, lt, g_sb[d][:, 512:1024],
                                     start=(d == 0), stop=(d == ND - 1))
                nc.scalar.activation(expT[sk][:, 0:512], ps0, Exp, scale=SCALE)
                nc.scalar.activation(expT[sk][:, 512:1024], ps1, Exp, scale=SCALE)

            # ---- phase 4: rowsum -> r_sb [128, 8] (col j: 1/rowsum, q-tile j)
            pc = qbank(0, "pcs", shape=(1, 1024))
            pc0, pc1 = pc[0:1, 0:512], pc[0:1, 512:1024]
            for sk in range(NS):
                nc.tensor.matmul(pc0, ones_sb[:], expT[sk][:, 0:512],
                                 start=(sk == 0), stop=(sk == NS - 1))
                nc.tensor.matmul(pc1, ones_sb[:], expT[sk][:, 512:1024],
                                 start=(sk == 0), stop=(sk == NS - 1))
            rs_row = ptile([1, SQ], "rs_row", "g0", dt=DT)
            nc.vector.tensor_copy(rs_row[0:1, 0:512], pc0)
            nc.vector.tensor_copy(rs_row[0:1, 512:1024], pc1)
            # transpose [1, 1024] -> [128, 8] via 8 K=1 fp32 matmuls
            psr = qbank(1, "psr", shape=(P, ND))
            one1 = ones_sb[0:1, 0:1].bitcast(DT)
            for j in range(ND):
                nc.tensor.matmul(psr[:, j:j + 1], rs_row[0:1, j * P:(j + 1) * P],
                                 one1, start=True, stop=True)
            r_sb = ptile([P, ND], "rsb", "rsb", dt=DT)
            nc.vector.reciprocal(r_sb[:], psr[:])

            # ---- phase 5: OT[e][128, SQ] accumulated in PSUM, 2 e-group passes
            ot = [ptile([P, SQ], f"ot_sb{e}", f"g{e}") for e in range(ND)]
            for eg in range(2):
                otp = [qbank(i, f"ot_{eg}_{i}") for i in range(4)]
                for sk in range(NS):
                    vb = ptile([P, D], f"vload_{eg}_{sk}", f"xt{sk % 6}")
                    nc.sync.dma_start(vb[:], v_spill[sk * P:(sk + 1) * P, :])
                    for i in range(4):
                        e = eg * 4 + i
                        lt = vb[:, e * P:(e + 1) * P]
                        nc.tensor.matmul(otp[i][:, 0:512], lt,
                                         expT[sk][:, 0:512],
                                         start=(sk == 0), stop=(sk == NS - 1))
                        nc.tensor.matmul(otp[i][:, 512:1024], lt,
                                         expT[sk][:, 512:1024],
                                         start=(sk == 0), stop=(sk == NS - 1))
                for i in range(4):
                    e = eg * 4 + i
                    nc.vector.tensor_copy(ot[e][:, 0:512], otp[i][:, 0:512])
                    nc.vector.tensor_copy(ot[e][:, 512:1024], otp[i][:, 512:1024])

            # ---- phase 6: proj y[g] = (OT.T @ wpT) * r + bias ----
            wp = load_wblk(wp_d)
            bias_sb = ptile([P, D], "bias", "vstage", bufs=3, dt=DT)
            nc.sync.dma_start(bias_sb[:], bias_d[:, :])
            for g in range(8):
                py = qbank(g % 4, f"py_{g}")
                ps0, ps1 = py[:, 0:512], py[:, 512:1024]
                for e in range(ND):
                    lt = ot[e][:, g * P:(g + 1) * P]
                    nc.tensor.matmul(ps0, lt, wp[e][:, 0:512],
                                     start=(e == 0), stop=(e == ND - 1))
                    nc.tensor.matmul(ps1, lt, wp[e][:, 512:1024],
                                     start=(e == 0), stop=(e == ND - 1))
                ysb = ptile([P, D], f"ysb_{g}", "vstage", bufs=3, dt=DT)
                rcol = r_sb[:, g:g + 1]
                nc.vector.scalar_tensor_tensor(ysb[:, 0:512], ps0, rcol,
                                               bias_sb[:, 0:512], MUL, ADD)
                nc.vector.scalar_tensor_tensor(ysb[:, 512:1024], ps1, rcol,
                                               bias_sb[:, 512:1024], MUL, ADD)
                nc.sync.dma_start(y_d[g * P:(g + 1) * P, :], ysb[:])

    nc.compile()
    return nc


def _get_nc():
    if "nc" not in _CACHED:
        _CACHED["nc"] = _build_nc()
    return _CACHED["nc"]


def make_in_maps(x, w_qkv, w_proj, b_proj):
    wq = w_qkv[0:D]
    wk = w_qkv[D:2 * D]
    w2 = wk.T @ wq                       # weight-only fold: scores = X W2 Xq^T
    w2T = np.ascontiguousarray(w2.T)
    wvT = np.ascontiguousarray(w_qkv[2 * D:3 * D].T)
    wpT = np.ascontiguousarray(w_proj.T)
    bias = np.ascontiguousarray(np.broadcast_to(b_proj[None, :], (P, D)))
    ones = np.ones((P, 1), dtype=np.float32)
    in_maps = []
    for c in range(8):
        b, h = c // 2, c % 2
        own = x[b, h * SQ:(h + 1) * SQ]       # [1024, D] our queries
        other = x[b, (1 - h) * SQ:(2 - h) * SQ]
        xt = np.ascontiguousarray(np.concatenate([own.T, other.T], axis=1))
        in_maps.append({
            "xt": xt, "w2t": w2T, "wvt": wvT, "wpt": wpT,
            "bias": bias, "ones": ones,
        })
    return in_maps


def gather_out(results):
    out = np.empty((4, S, D), dtype=np.float32)
    for c in range(8):
        b, h = c // 2, c % 2
        out[b, h * SQ:(h + 1) * SQ] = results[c]["y"]
    return out


def kernel(x, w_qkv, w_proj, b_proj):
    from concourse import bass_utils
    nc = _get_nc()
    in_maps = make_in_maps(np.asarray(x, dtype=np.float32),
                           np.asarray(w_qkv, dtype=np.float32),
                           np.asarray(w_proj, dtype=np.float32),
                           np.asarray(b_proj, dtype=np.float32))
    res = bass_utils.run_bass_kernel_spmd(nc, in_maps, list(range(8))).results
    return gather_out(res)
